# revision 7
# baseline (speedup 1.0000x reference)
"""DCCA 2D loss kernel for 8 Trainium2 NeuronCores (Bass/Tile).

Strategy (data-parallel over the m = B*C = 2048 sample axis):
  - Each core gets 256 samples of both views. Per 16-sample block it loads
    both views with 128-partition DMAs (partition = (sample-in-block)*8 +
    n//8, so every DMA descriptor reads a contiguous 4KB row run), PE-
    transposes the [128,128] slices, assembles per-sample fused tiles
    T_m = [H1_m^T | H2_m^T] (the n-axis lands in a fixed permutation,
    under which the final scalar is exactly invariant), and accumulates
    the fused Gram  G += T_m^T T_m  in PSUM.  G's 64x64 blocks are
    [[S11raw, S12raw], [S21raw, S22raw]].
  - A dependency-free warm-up AllReduce is triggered as the very first
    gpsimd instruction so the ~45us ncfw bootstrap barrier overlaps the
    main load loop; the real 64KB AllReduce(add) then starts immediately
    when the partial Gram is staged, and lands the summed G on all cores
    (no gather + tree-sum needed).
  - Replicated epilogue without eigh:  with  S = c1*G + R*I  and
    A = blockdiag(S11, S22),   corr^2 = trace(S11^-1 S12 S22^-1 S12^T)
    = sum( (A^-1 S)[0:64,64:128] * (S A^-1)[0:64,64:128] ).
    A^-1 = s*Y with s = 128/tr(A) and Y from scaled Newton-Schulz
    Y1 = 2I - s*A;  Y_{t+1} = 2*Y_t - Y_t*(s*A)*Y_t  (all terms are
    polynomials in A, hence symmetric; cond(A)~1.07 so Y2 is exact to
    ~1e-6).  corr = s * sqrt(sum((Y S)_12 * (S Y)_12)); output -corr.
"""

import os

import numpy as np

# ---------------------------------------------------------------- constants
B, C, N, K = 32, 64, 64, 128
M = B * C                    # 2048 samples
NC = 8                       # cores
NS = M // NC                 # 256 samples per core
# Chunk schedule (samples per DMA chunk, per view). The first chunk is small
# so the PE can start within a few us instead of waiting for a full 2MB
# SWDGE descriptor generation.
CHUNKS = (16, 32, 32, 32, 32, 32, 32, 32, 16)
assert sum(CHUNKS) == NS
R_RIDGE = 1e-4
C1 = float((1.0 - 1.0 / M) ** 2 / (M * (M - 1)))  # Gram -> Sigma scale

# "bf16" (fast, ~1e-4 rel err), "f32r" (reduced fp32 matmul), "f32" (safest)
GRAM_MODE = os.environ.get("BASS_GRAM_MODE", "bf16")
# Newton-Schulz steps after Y1 = 2I - sA (1 -> inverse err ~1e-6)
EPI_ITERS = int(os.environ.get("BASS_EPI_ITERS", "1"))
PIPELINE = os.environ.get("BASS_PIPELINE", "1") == "1"
# "ar": AllReduce the 64KB partial Gram. "ag": AllGather + on-core tree sum.
CC_MODE = os.environ.get("BASS_CC_MODE", "ar")
# Samples per partition-block. G=16 -> 4KB DMA read descriptors; needs the
# bf16 PSUM transpose tile ([128, 8, 128] bf16 = one 2KB bank). The f32
# fallbacks use G=8 so the transpose tile still fits one bank.
G = 16 if GRAM_MODE == "bf16" else 8

_CACHE = {}


def _gdt(mybir):
    return {
        "bf16": mybir.dt.bfloat16,
        "f32r": mybir.dt.float32r,
        "f32": mybir.dt.float32,
    }[GRAM_MODE]


def _gnp():
    if GRAM_MODE == "bf16":
        import ml_dtypes

        return ml_dtypes.bfloat16
    return np.float32


def _build():
    import concourse.bass as bass
    import concourse.mybir as mybir
    import concourse.tile as tile
    from concourse import bacc

    gdt = _gdt(mybir)
    f32 = mybir.dt.float32

    nc = bacc.Bacc(
        "TRN2",
        target_bir_lowering=False,
        debug=False,
        enable_asserts=False,
        num_devices=NC,
    )

    x1 = nc.dram_tensor("x1", [NS, N, K], f32, kind="ExternalInput").ap()
    x2 = nc.dram_tensor("x2", [NS, N, K], f32, kind="ExternalInput").ap()
    ident_d = nc.dram_tensor("ident", [128, 128], gdt, kind="ExternalInput").ap()
    eye2_d = nc.dram_tensor("eye2", [128, 128], f32, kind="ExternalInput").ap()
    reye_d = nc.dram_tensor("reye", [128, 128], f32, kind="ExternalInput").ap()
    maskc_d = nc.dram_tensor("maskc", [128, 128], f32, kind="ExternalInput").ap()
    eyei_d = nc.dram_tensor("eyei", [128, 128], f32, kind="ExternalInput").ap()
    ones_d = nc.dram_tensor("onesf", [128, 128], f32, kind="ExternalInput").ap()
    warm_d = nc.dram_tensor("warm", [1, 16], f32, kind="ExternalInput").ap()
    out_d = nc.dram_tensor("out", [1, 1], f32, kind="ExternalOutput").ap()

    groups = [list(range(NC))]

    with tile.TileContext(nc) as tc:
        import contextlib

        with contextlib.ExitStack() as ctx:
            # Dependency-free warm-up collective, first gpsimd instruction:
            # starts the ncfw bootstrap (~45us barrier) at t~0 so it fully
            # overlaps the main load loop and the real AllReduce is warm.
            dwpool = ctx.enter_context(tc.tile_pool(name="dwarm", bufs=1, space="DRAM"))
            win = dwpool.tile([1, 16], f32)
            wout = dwpool.tile([1, 16], f32)
            nc.gpsimd.dma_start(win[:], warm_d)
            nc.gpsimd.collective_compute(
                "AllReduce",
                mybir.AluOpType.add,
                replica_groups=groups,
                ins=[win.opt()],
                outs=[wout.opt()],
            )

            cpool = ctx.enter_context(tc.tile_pool(name="consts", bufs=1))
            ident = cpool.tile([128, 128], gdt)
            nc.sync.dma_start(ident[:], ident_d)
            eye2 = cpool.tile([128, 128], f32)
            nc.sync.dma_start(eye2[:], eye2_d)
            reye = cpool.tile([128, 128], f32)
            nc.sync.dma_start(reye[:], reye_d)
            maskc = cpool.tile([128, 128], f32)
            nc.sync.dma_start(maskc[:], maskc_d)
            eyei = cpool.tile([128, 128], f32)
            nc.sync.dma_start(eyei[:], eyei_d)
            onesf = cpool.tile([128, 128], f32)
            nc.sync.dma_start(onesf[:], ones_d)

            spool = ctx.enter_context(tc.tile_pool(name="work", bufs=2))
            gsb = spool.tile([128, 128], f32, tag="gsb")

            # ---------------- main loop: per-core partial fused Gram ------
            # Layout per chunk/view SBUF tile V [128, CH/G, G//2 * 128]:
            #   V[16h+u, j, r*128+k] = X[s0+G*j+h, 4u+r, k]
            # so each DMA descriptor reads (G/2) consecutive n-rows = 4KB.
            # Per G-sample block: 8 transposes (2 views x 4 r) emitted as
            # regular matmuls against the identity (keeps PE-HAM warm and
            # enables FWL for bf16, unlike transpose-mode), one assembly
            # copy per view into the fused TT tile, then G Gram matmuls.
            with (
                tc.tile_pool(name="vload", bufs=4) as vpool,
                tc.tile_pool(name="ttp", bufs=3) as ttpool,
                tc.tile_pool(name="ptp", bufs=1, space="PSUM") as ptpool,
                tc.tile_pool(name="gpp", bufs=1, space="PSUM") as gpool,
            ):
                if GRAM_MODE == "f32r":
                    gpA = gpool.tile([128, 256], f32, tag="gpA")
                    gpB = gpool.tile([128, 256], f32, tag="gpB")
                else:
                    gp = gpool.tile([128, 128], f32, tag="gp")

                first = [True, True]  # start-flags for (gpA, gpB) / (gp,)
                n_blocks_total = NS // G

                def emit_gram(tt8, bi):
                    last = bi == n_blocks_total - 1
                    tt8f = tt8.rearrange("p h b u -> p (h b u)")
                    if GRAM_MODE == "f32r":
                        for h in range(G):
                            acc = gpA if h % 2 == 0 else gpB
                            fi = h % 2
                            nc.tensor.matmul(
                                acc[:],
                                tt8f[:, 128 * h : 128 * (h + 1)],
                                tt8f[:, 256 * (h // 2) : 256 * (h // 2 + 1)],
                                start=first[fi],
                                stop=last and h >= G - 2,
                            )
                            first[fi] = False
                    else:
                        for h in range(G):
                            nc.tensor.matmul(
                                gp[:],
                                tt8f[:, 128 * h : 128 * (h + 1)],
                                tt8f[:, 128 * h : 128 * (h + 1)],
                                start=first[0],
                                stop=last and h == G - 1,
                            )
                            first[0] = False

                pending = None  # (tt, block_index) awaiting Gram matmuls
                bi = 0
                s0 = 0
                NR = G // 2      # r-values (n-rows per DMA run)
                NU = 128 // G    # u-values (partition n-groups)
                for ci, CH in enumerate(CHUNKS):
                    nj = CH // G
                    vts = []
                    for vi, xsrc in enumerate((x1, x2)):
                        vt = vpool.tile([128, nj, NR * 128], gdt, tag=f"v{vi}")
                        src = xsrc[s0 : s0 + CH].rearrange(
                            "(j h) (u r) k -> (h u) j (r k)", h=G, r=NR
                        )
                        if gdt == f32:
                            nc.sync.dma_start(vt[:], src)
                        else:
                            # SWDGE casts f32 -> bf16 (or relabels f32r)
                            # during the transfer.
                            nc.gpsimd.dma_start(vt[:], src)
                        vts.append(vt)
                    s0 += CH

                    for j in range(nj):
                        tt = ttpool.tile([128, G, 2, 64], gdt, tag="tt")
                        for vi in range(2):
                            ptdt = f32 if GRAM_MODE == "bf16" else gdt
                            ptv = ptpool.tile([128, NR, 128], ptdt, tag=f"pt{vi}")
                            for r in range(NR):
                                if GRAM_MODE == "bf16":
                                    nc.tensor.matmul(
                                        ptv[:, r, :],
                                        vts[vi][:, j, r * 128 : (r + 1) * 128],
                                        ident[:],
                                        start=True,
                                        stop=True,
                                    )
                                else:
                                    nc.tensor.transpose(
                                        ptv[:, r, :],
                                        vts[vi][:, j, r * 128 : (r + 1) * 128],
                                        ident[:],
                                    )
                            nc.any.tensor_copy(
                                out=tt[:, :, vi, :].rearrange(
                                    "p h (r u) -> p h r u", r=NR
                                ),
                                in_=ptv.rearrange(
                                    "p r (h u) -> p h r u", h=G
                                ),
                            )
                        # one-block software pipeline: this block's Gram
                        # matmuls are emitted after the NEXT block's
                        # transposes, so the PE never stalls on the copy.
                        if PIPELINE:
                            if pending is not None:
                                emit_gram(*pending)
                            pending = (tt, bi)
                        else:
                            emit_gram(tt, bi)
                        bi += 1
                if pending is not None:
                    emit_gram(*pending)

                if GRAM_MODE == "f32r":
                    nc.vector.tensor_add(gsb[:], gpA[:, 0:128], gpB[:, 128:256])
                else:
                    nc.vector.tensor_copy(gsb[:], gp[:])

            # ---------------- AllReduce + replicated epilogue -------------
            with (
                tc.tile_pool(name="dram", bufs=1, space="DRAM") as dpool,
                tc.tile_pool(name="epp", bufs=1, space="PSUM") as epool,
            ):
                din = dpool.tile([128, 128], f32)
                nc.gpsimd.dma_start(din[:], gsb[:])
                Sg = spool.tile([128, 128], f32, tag="Sg")
                if CC_MODE == "ar":
                    dout = dpool.tile([128, 128], f32)
                    nc.gpsimd.collective_compute(
                        "AllReduce",
                        mybir.AluOpType.add,
                        replica_groups=groups,
                        ins=[din.opt()],
                        outs=[dout.opt()],
                    )
                    nc.gpsimd.dma_start(Sg[:], dout[:])
                else:
                    dout = dpool.tile([NC, 128, 128], f32)
                    nc.gpsimd.collective_compute(
                        "AllGather",
                        mybir.AluOpType.bypass,
                        replica_groups=groups,
                        ins=[din.opt()],
                        outs=[dout.opt()],
                    )
                    gall = spool.tile([128, NC, 128], f32, tag="gall")
                    nc.sync.dma_start(gall[:], dout[:].rearrange("c p k -> p c k"))
                    g4 = spool.tile([128, 4, 128], f32, tag="g4")
                    nc.vector.tensor_add(g4[:], gall[:, 0:4, :], gall[:, 4:8, :])
                    g2 = spool.tile([128, 2, 128], f32, tag="g2")
                    nc.vector.tensor_add(g2[:], g4[:, 0:2, :], g4[:, 2:4, :])
                    nc.vector.tensor_add(Sg[:], g2[:, 0, :], g2[:, 1, :])

                # s = 128/tr(A) broadcast to all partitions in one matmul:
                # tr chain: diag mask -> row-reduce -> ones^T . dcol.
                dg = spool.tile([128, 128], f32, tag="dg")
                nc.vector.tensor_mul(dg[:], Sg[:], eyei[:])
                dcol = spool.tile([128, 1], f32, tag="dcol")
                nc.vector.reduce_sum(dcol[:], dg[:], axis=mybir.AxisListType.X)
                trp = epool.tile([128, 1], f32, tag="trp")
                nc.tensor.matmul(trp[:], onesf[:], dcol[:], start=True, stop=True)
                # tr(A) = C1*tr(G) + 128*R  ->  s = 1/(tr(A)/128)
                tsc = spool.tile([128, 1], f32, tag="tsc")
                nc.vector.tensor_scalar(
                    tsc[:], trp[:], C1 / 128.0, R_RIDGE,
                    op0=mybir.AluOpType.mult, op1=mybir.AluOpType.add,
                )
                scol = spool.tile([128, 1], f32, tag="scol")
                nc.vector.reciprocal(scol[:], tsc[:])

                # An = -s*A  with  A = C1*(G o blockmask) + R*I
                Am = spool.tile([128, 128], f32, tag="Am")
                nc.vector.tensor_mul(Am[:], Sg[:], maskc[:])
                Aa = spool.tile([128, 128], f32, tag="Aa")
                nc.vector.tensor_add(Aa[:], Am[:], reye[:])
                An = spool.tile([128, 128], f32, tag="An")
                nc.vector.tensor_scalar(
                    An[:], Aa[:], scol[:], -1.0,
                    op0=mybir.AluOpType.mult, op1=mybir.AluOpType.mult,
                )

                # Y1 = 2I - s*A ; then Y <- 2Y + Y*(An*Y)
                ycur = spool.tile([128, 128], f32, tag="yn")
                nc.vector.tensor_add(ycur[:], eye2[:], An[:])

                # S = C1*G + R*I (off critical path; used only at the end)
                S = spool.tile([128, 128], f32, tag="S")
                nc.vector.tensor_scalar_mul(S[:], Sg[:], C1)
                nc.vector.tensor_add(S[:], S[:], reye[:])

                for _ in range(EPI_ITERS):
                    pp = epool.tile([128, 128], f32, tag="pp")
                    nc.tensor.matmul(pp[:], An[:], ycur[:], start=True, stop=True)
                    ps = spool.tile([128, 128], f32, tag="ps")
                    nc.any.tensor_copy(ps[:], pp[:])
                    yp = epool.tile([128, 128], f32, tag="yp")
                    nc.tensor.matmul(yp[:], eye2[:], ycur[:], start=True, stop=False)
                    nc.tensor.matmul(yp[:], ycur[:], ps[:], start=False, stop=True)
                    ynew = spool.tile([128, 128], f32, tag="yn")
                    nc.vector.tensor_copy(ynew[:], yp[:])
                    ycur = ynew

                up = epool.tile([128, 128], f32, tag="up")
                nc.tensor.matmul(up[:], ycur[:], S[:], start=True, stop=True)
                vp = epool.tile([128, 128], f32, tag="vp")
                nc.tensor.matmul(vp[:], S[:], ycur[:], start=True, stop=True)
                us = spool.tile([64, 64], f32, tag="us")
                nc.vector.tensor_copy(us[:], up[0:64, 64:128])
                pm = spool.tile([64, 64], f32, tag="pm")
                nc.vector.tensor_tensor(
                    pm[:], us[:], vp[0:64, 64:128], mybir.AluOpType.mult
                )
                pcol = spool.tile([64, 1], f32, tag="pcol")
                nc.vector.reduce_sum(pcol[:], pm[:], axis=mybir.AxisListType.X)
                cp = epool.tile([1, 1], f32, tag="cp")
                nc.tensor.matmul(cp[:], pcol[:], onesf[0:64, 0:1], start=True, stop=True)
                c2 = spool.tile([1, 1], f32, tag="c2")
                nc.vector.tensor_copy(c2[:], cp[:])
                root = spool.tile([1, 1], f32, tag="root")
                nc.scalar.sqrt(root[:], c2[:])
                # out = -s * sqrt(corr^2)
                oroot = spool.tile([1, 1], f32, tag="oroot")
                nc.vector.tensor_scalar(
                    oroot[:], root[:], scol[0:1, 0:1], -1.0,
                    op0=mybir.AluOpType.mult, op1=mybir.AluOpType.mult,
                )
                nc.gpsimd.dma_start(out_d, oroot[:])

    nc.compile()
    return nc


def _get_nc():
    key = (GRAM_MODE, EPI_ITERS, CC_MODE)
    if key not in _CACHE:
        _CACHE[key] = _build()
    return _CACHE[key]


def _const_inputs():
    eye = np.eye(128, dtype=np.float32)
    maskd = np.zeros((128, 128), dtype=np.float32)
    maskd[:64, :64] = np.eye(64, dtype=np.float32)
    maskd[64:, 64:] = np.eye(64, dtype=np.float32)
    return {
        "ident": np.eye(128).astype(_gnp()),
        "eye2": (2.0 * eye).astype(np.float32),
        "reye": (R_RIDGE * eye).astype(np.float32),
        "maskc": (C1 * maskd).astype(np.float32),
        "eyei": eye,
        "onesf": np.ones((128, 128), dtype=np.float32),
        "warm": np.zeros((1, 16), dtype=np.float32),
    }


def kernel(data_view1, data_view2):
    from concourse import bass_utils

    h1 = np.ascontiguousarray(data_view1, dtype=np.float32).reshape(M, N, K)
    h2 = np.ascontiguousarray(data_view2, dtype=np.float32).reshape(M, N, K)

    consts = _const_inputs()
    in_maps = []
    for c in range(NC):
        m = {
            "x1": h1[c * NS : (c + 1) * NS],
            "x2": h2[c * NS : (c + 1) * NS],
        }
        m.update(consts)
        in_maps.append(m)

    nc = _get_nc()
    trace = os.environ.get("BASS_KERNEL_TRACE", "0") == "1"
    res = bass_utils.run_bass_kernel_spmd(
        nc, in_maps, core_ids=list(range(NC)), trace=trace
    )
    if trace:
        kernel.last_results = res
    val = np.asarray(res.results[0]["out"]).reshape(())
    return val.astype(np.float32)
